# revision 3
# baseline (speedup 1.0000x reference)
"""Trainium2 Bass kernel for nn_PerformerAttention.

reference math (B,H,S,D = 4,8,2048,64):
    qf = relu(q @ W.T); kf = relu(k @ W.T)          # [B,H,S,D]
    scores = qf @ kf.T                              # [B,H,S,S]
    attn_weights = softmax(scores, axis=-1)
    attn_output  = v * rowsum(attn_weights) == v    # softmax rows sum to 1
    returns (attn_output, attn_weights)

Sharding: B*H = 32 (b,h) pairs, 4 per core across 8 cores.  Host-side
layout prep only: q/k are transposed to [.., D, S] so the device never
needs on-chip transposes (matmul contracts over the partition dim).

Per-core device program (per head):
    feature transform qfT/kfT = relu(W^T.T @ qT/kT) on the PE,
    written to both SBUF partition halves (PE column tiling) so score
    matmuls can row-pack the half-empty K=64 PE array.  Optionally the
    features are split hi/lo bf16 so each score matmul is 3 bf16 passes
    (exact products; only the lo*lo term is dropped, ~1e-3 rel err)
    instead of 4 fp32 half-passes.
    for each 128-row tile of scores:
        s = qfT_chunk.T @ kfT           # PSUM [128, S] fp32, row-packed
        fused PSUM->SBUF copy + row-max (VectorE), exp+row-sum (ScalarE),
        multiply by 1/sum (engine per cfg), DMA out.
"""

import os
import numpy as np

B, H, S, D = 4, 8, 2048, 64
NCORES = 8
HPC = (B * H) // NCORES      # heads per core = 4
PAIRS = HPC // 2             # head pairs per core = 2
RT = S // 128                # 128-row score tiles per head = 16
NCH = S // 512               # 512-col matmul chunks per row tile = 4

# Tunables (test.py may override before calling kernel()).
CONFIG = {
    "pack": True,         # row-pack score matmuls across PE partition halves
    "copymax": True,      # fused PSUM->SBUF copy + row-max on VectorE
    "score_bf16": True,   # 3-term bf16-split score matmuls (else fp32 4-pass)
    "mul_pattern": "g",   # normalize-mul engine per tile, cycled: g/v/s
    "trace": False,       # request NTFF profile from the run
}

_CACHE = {}


def _build_program(cfg):
    from contextlib import ExitStack

    import concourse.bacc as bacc
    import concourse.mybir as mybir
    import concourse.tile as tile

    f32 = mybir.dt.float32
    bf16 = mybir.dt.bfloat16
    AF = mybir.ActivationFunctionType
    OP = mybir.AluOpType
    AX = mybir.AxisListType

    nc = bacc.Bacc(
        "TRN2",
        target_bir_lowering=False,
        debug=False,
        num_devices=NCORES,
    )

    qt = nc.dram_tensor("qt", [PAIRS * 128, S], f32, kind="ExternalInput").ap()
    kt = nc.dram_tensor("kt", [PAIRS * 128, S], f32, kind="ExternalInput").ap()
    wt = nc.dram_tensor("wt", [128, D], f32, kind="ExternalInput").ap()
    out = nc.dram_tensor("out", [HPC * S, S], f32, kind="ExternalOutput").ap()

    with tile.TileContext(nc) as tc, ExitStack() as ctx:
        const = ctx.enter_context(tc.tile_pool(name="const", bufs=1))
        inp = ctx.enter_context(tc.tile_pool(name="inp", bufs=2))
        feat = ctx.enter_context(tc.tile_pool(name="feat", bufs=2))
        psum = ctx.enter_context(tc.tile_pool(name="psum", bufs=2, space="PSUM"))
        work = ctx.enter_context(tc.tile_pool(name="work", bufs=3))
        stat = ctx.enter_context(tc.tile_pool(name="stat", bufs=8))
        outp = ctx.enter_context(tc.tile_pool(name="outp", bufs=4))

        wtt = const.tile([128, D], f32, tag="wtt")
        nc.sync.dma_start(wtt[:], wt[:, :])

        tile_idx = 0

        def softmax_tail(ps, h, m):
            """PSUM scores tile -> normalized SBUF tile -> DMA out."""
            nonlocal tile_idx
            negmax = stat.tile([128, 1], f32, tag="negmax")
            rowsum = stat.tile([128, 1], f32, tag="rowsum")
            rinv = stat.tile([128, 1], f32, tag="rinv")
            expt = work.tile([128, S], f32, tag="expt")
            if cfg["copymax"]:
                # sc = -scores (SBUF copy), negmax = min(-scores) = -rowmax.
                # Frees the PSUM tile after this single VectorE pass.
                sc = work.tile([128, S], f32, tag="sc")
                nc.vector.tensor_scalar(
                    sc[:], ps[:], -1.0, None, OP.mult, OP.min, accum_out=negmax[:]
                )
                # exp(-1*sc + negmax) = exp(s - max); fused row-sum.
                nc.scalar.activation(
                    expt[:], sc[:], AF.Exp,
                    bias=negmax[:], scale=-1.0, accum_out=rowsum[:],
                )
            else:
                nc.vector.reduce_max(negmax[:], ps[:], AX.X, negate=True)
                nc.scalar.activation(
                    expt[:], ps[:], AF.Exp,
                    bias=negmax[:], scale=1.0, accum_out=rowsum[:],
                )
            nc.vector.reciprocal(rinv[:], rowsum[:])
            ot = outp.tile([128, S], f32, tag="ot")
            eng = cfg["mul_pattern"][tile_idx % len(cfg["mul_pattern"])]
            if eng == "s":
                nc.scalar.activation(ot[:], expt[:], AF.Copy, bias=0.0, scale=rinv[:])
            elif eng == "g":
                nc.gpsimd.tensor_scalar(ot[:], expt[:], rinv[:], None, OP.mult, OP.bypass)
            else:
                nc.vector.tensor_scalar(ot[:], expt[:], rinv[:], None, OP.mult, OP.bypass)
            nc.sync.dma_start(out[h * S + 128 * m : h * S + 128 * (m + 1), :], ot[:])
            tile_idx += 1

        for p in range(PAIRS):
            qtt = inp.tile([128, S], f32, tag="qtt")
            nc.sync.dma_start(qtt[:], qt[128 * p : 128 * (p + 1), :])
            ktt = inp.tile([128, S], f32, tag="ktt")
            nc.sync.dma_start(ktt[:], kt[128 * p : 128 * (p + 1), :])
            for e in range(2):
                h = 2 * p + e
                rb = 64 * e  # partition base of this head's qT/kT rows
                # ---- feature transform, duplicated across partition halves
                if cfg["score_bf16"]:
                    qhi = feat.tile([128, S], bf16, tag="qhi")
                    qlo = feat.tile([128, S], bf16, tag="qlo")
                    khi = feat.tile([128, S], bf16, tag="khi")
                    klo = feat.tile([128, S], bf16, tag="klo")
                    srcs = ((qtt, qhi, qlo), (ktt, khi, klo))
                else:
                    qf = feat.tile([128, S], f32, tag="qhi")
                    kf = feat.tile([128, S], f32, tag="khi")
                    srcs = ((qtt, qf, None), (ktt, kf, None))
                for src, hi, lo in srcs:
                    pf = psum.tile([128, S], f32, tag="ps")
                    for j in range(NCH):
                        cs = slice(512 * j, 512 * (j + 1))
                        for c in (0, 64):
                            nc.tensor.matmul(
                                pf[c : c + 64, cs],
                                lhsT=wtt[rb : rb + 64, :],
                                rhs=src[rb : rb + 64, cs],
                                start=True, stop=True,
                                tile_position=(rb, c),
                            )
                    nc.scalar.activation(hi[:], pf[:], AF.Relu)
                    if lo is not None:
                        # lo = relu(pf) - hi, rounded to bf16 (the correction
                        # term the bf16 hi dropped).
                        nc.vector.scalar_tensor_tensor(
                            lo[:], pf[:], 0.0, hi[:], OP.max, OP.subtract
                        )

                # ---- scores + softmax
                def score_mms(ps_t, m, base, j):
                    cs = slice(512 * j, 512 * (j + 1))
                    mc = slice(128 * m, 128 * (m + 1))
                    if cfg["score_bf16"]:
                        terms = (
                            (qhi, khi, True, False),
                            (qhi, klo, False, False),
                            (qlo, khi, False, True),
                        )
                        for ql, kl, st, sp in terms:
                            nc.tensor.matmul(
                                ps_t[:, cs],
                                lhsT=ql[base : base + 64, mc],
                                rhs=kl[base : base + 64, cs],
                                start=st, stop=sp,
                                tile_position=(base, 0),
                            )
                    else:
                        nc.tensor.matmul(
                            ps_t[:, cs],
                            lhsT=qf[base : base + 64, mc],
                            rhs=kf[base : base + 64, cs],
                            start=True, stop=True,
                            tile_position=(base, 0),
                        )

                if cfg["pack"]:
                    # Interleave two row tiles on opposite PE halves so the
                    # K=64 matmuls run concurrently (distinct row groups).
                    for mp in range(RT // 2):
                        m0, m1 = 2 * mp, 2 * mp + 1
                        ps0 = psum.tile([128, S], f32, tag="ps")
                        ps1 = psum.tile([128, S], f32, tag="ps")
                        for j in range(NCH):
                            score_mms(ps0, m0, 0, j)
                            score_mms(ps1, m1, 64, j)
                        softmax_tail(ps0, h, m0)
                        softmax_tail(ps1, h, m1)
                else:
                    for m in range(RT):
                        ps = psum.tile([128, S], f32, tag="ps")
                        for j in range(NCH):
                            score_mms(ps, m, 0, j)
                        softmax_tail(ps, h, m)

    nc.compile()
    return nc


def _cfg_key(cfg):
    return (cfg["pack"], cfg["copymax"], cfg["score_bf16"], cfg["mul_pattern"])


def _get_program(cfg):
    key = _cfg_key(cfg)
    if key not in _CACHE:
        _CACHE[key] = _build_program(cfg)
    return _CACHE[key]


def make_in_maps(q, k, random_weights):
    """Host-side sharding/layout prep -> per-core input dicts."""
    q = np.asarray(q, dtype=np.float32)
    k = np.asarray(k, dtype=np.float32)
    w = np.asarray(random_weights, dtype=np.float32)
    # [B,H,S,D] -> [B*H, D, S]
    qT = np.ascontiguousarray(q.transpose(0, 1, 3, 2)).reshape(B * H, D, S)
    kT = np.ascontiguousarray(k.transpose(0, 1, 3, 2)).reshape(B * H, D, S)
    wt = np.ascontiguousarray(np.concatenate([w.T, w.T], axis=0))  # [128, D]
    in_maps = []
    for c in range(NCORES):
        qc = np.ascontiguousarray(qT[HPC * c : HPC * (c + 1)]).reshape(PAIRS * 128, S)
        kc = np.ascontiguousarray(kT[HPC * c : HPC * (c + 1)]).reshape(PAIRS * 128, S)
        in_maps.append({"qt": qc, "kt": kc, "wt": wt})
    return in_maps


def run_device(q, k, random_weights, cfg=None, trace=None):
    """Compile (cached), run on all 8 cores, return (attn_weights, results)."""
    from concourse.bass_utils import run_bass_kernel_spmd

    cfg = dict(CONFIG if cfg is None else cfg)
    if trace is not None:
        cfg["trace"] = trace
    nc = _get_program(cfg)
    in_maps = make_in_maps(q, k, random_weights)
    res = run_bass_kernel_spmd(
        nc, in_maps, core_ids=list(range(NCORES)), trace=cfg["trace"]
    )
    outs = [res.results[c]["out"].reshape(HPC, S, S) for c in range(NCORES)]
    attn_weights = np.concatenate(outs, axis=0).reshape(B, H, S, S)
    return attn_weights, res


def kernel(q, k, v, random_weights):
    attn_weights, _ = run_device(q, k, random_weights)
    attn_output = np.asarray(v, dtype=np.float32)
    return attn_output, attn_weights


# revision 4
# speedup vs baseline: 4.2554x; 4.2554x over previous
"""Trainium2 Bass kernel for nn_PerformerAttention.

reference math (B,H,S,D = 4,8,2048,64):
    qf = relu(q @ W.T); kf = relu(k @ W.T)          # [B,H,S,D]
    scores = qf @ kf.T                              # [B,H,S,S]
    attn_weights = softmax(scores, axis=-1)
    attn_output  = v * rowsum(attn_weights) == v    # softmax rows sum to 1
    returns (attn_output, attn_weights)

Sharding: B*H = 32 (b,h) pairs, 4 per core across 8 cores.  Host-side
layout prep only: q/k are transposed to [.., D, S] so the device never
needs on-chip transposes (matmul contracts over the partition dim).

Per-core device program (per head):
    feature transform qfT/kfT = relu(W^T.T @ qT/kT) on the PE, written
    to both SBUF partition halves (PE column tiling) so score matmuls
    can row-pack the half-empty K=64 PE array (fp32 matmul streams 4
    passes but drains once, so two row-groups overlap ~2x; bf16 is
    drain-bound and does not pack).  Feature work for head h+1 is
    emitted in the middle of head h's score loop to hide the serial
    feature->relu chain.
    for each 128-row tile of scores:
        s = qfT_chunk.T @ kfT            # PSUM [128, S] fp32, row-packed
        fused PSUM->SBUF copy + row-max (VectorE), exp+row-sum (ScalarE),
        multiply by 1/sum (engine per cfg), DMA out.
"""

import os
import numpy as np

B, H, S, D = 4, 8, 2048, 64
NCORES = 8
HPC = (B * H) // NCORES      # heads per core = 4
PAIRS = HPC // 2             # head pairs per core = 2
RT = S // 128                # 128-row score tiles per head = 16
NCH = S // 512               # 512-col matmul chunks per row tile = 4

# Tunables (test.py may override before calling kernel()).
CONFIG = {
    "pack": True,         # row-pack score matmuls across PE partition halves
    "copymax": True,      # fused PSUM->SBUF copy + row-max on VectorE
    "score_bf16": False,  # 3-term bf16-split score matmuls (else fp32 4-pass)
    "mul_pattern": "vvs", # normalize-mul engine per tile, cycled: g/v/s
    "interleave_feat": True,  # emit head h+1 features inside head h's scores
    "trace": False,       # request NTFF profile from the run
}

_CACHE = {}


def _build_program(cfg):
    from contextlib import ExitStack

    import concourse.bacc as bacc
    import concourse.mybir as mybir
    import concourse.tile as tile

    f32 = mybir.dt.float32
    bf16 = mybir.dt.bfloat16
    AF = mybir.ActivationFunctionType
    OP = mybir.AluOpType
    AX = mybir.AxisListType

    nc = bacc.Bacc(
        "TRN2",
        target_bir_lowering=False,
        debug=False,
        num_devices=NCORES,
    )

    qt = nc.dram_tensor("qt", [PAIRS * 128, S], f32, kind="ExternalInput").ap()
    kt = nc.dram_tensor("kt", [PAIRS * 128, S], f32, kind="ExternalInput").ap()
    wt = nc.dram_tensor("wt", [128, D], f32, kind="ExternalInput").ap()
    out = nc.dram_tensor("out", [HPC * S, S], f32, kind="ExternalOutput").ap()

    with tile.TileContext(nc) as tc, ExitStack() as ctx:
        const = ctx.enter_context(tc.tile_pool(name="const", bufs=1))
        inp = ctx.enter_context(tc.tile_pool(name="inp", bufs=2))
        feat = ctx.enter_context(tc.tile_pool(name="feat", bufs=2))
        psum = ctx.enter_context(tc.tile_pool(name="psum", bufs=2, space="PSUM"))
        work = ctx.enter_context(tc.tile_pool(name="work", bufs=3))
        stat = ctx.enter_context(tc.tile_pool(name="stat", bufs=8))
        outp = ctx.enter_context(tc.tile_pool(name="outp", bufs=4))

        wtt = const.tile([128, D], f32, tag="wtt")
        nc.sync.dma_start(wtt[:], wt[:, :])

        tile_idx = 0

        def softmax_tail(ps, h, m):
            """PSUM scores tile -> normalized SBUF tile -> DMA out."""
            nonlocal tile_idx
            negmax = stat.tile([128, 1], f32, tag="negmax")
            rowsum = stat.tile([128, 1], f32, tag="rowsum")
            rinv = stat.tile([128, 1], f32, tag="rinv")
            expt = work.tile([128, S], f32, tag="expt")
            if cfg["copymax"]:
                # sc = -scores (SBUF copy), negmax = min(-scores) = -rowmax.
                # Frees the PSUM tile after this single VectorE pass.
                sc = work.tile([128, S], f32, tag="sc")
                nc.vector.tensor_scalar(
                    sc[:], ps[:], -1.0, None, OP.mult, OP.min, accum_out=negmax[:]
                )
                # exp(-1*sc + negmax) = exp(s - max); fused row-sum.
                nc.scalar.activation(
                    expt[:], sc[:], AF.Exp,
                    bias=negmax[:], scale=-1.0, accum_out=rowsum[:],
                )
            else:
                nc.vector.reduce_max(negmax[:], ps[:], AX.X, negate=True)
                nc.scalar.activation(
                    expt[:], ps[:], AF.Exp,
                    bias=negmax[:], scale=1.0, accum_out=rowsum[:],
                )
            nc.vector.reciprocal(rinv[:], rowsum[:])
            ot = outp.tile([128, S], f32, tag="ot")
            eng = cfg["mul_pattern"][tile_idx % len(cfg["mul_pattern"])]
            if eng == "s":
                nc.scalar.activation(ot[:], expt[:], AF.Copy, bias=0.0, scale=rinv[:])
            elif eng == "g":
                nc.gpsimd.tensor_scalar(ot[:], expt[:], rinv[:], None, OP.mult, OP.bypass)
            else:
                nc.vector.tensor_scalar(ot[:], expt[:], rinv[:], None, OP.mult, OP.bypass)
            nc.sync.dma_start(out[h * S + 128 * m : h * S + 128 * (m + 1), :], ot[:])
            tile_idx += 1

        # ---- per-head feature state -------------------------------------
        qtts, ktts = {}, {}
        feats = {}  # h -> (qhi, qlo, khi, klo) or (qf, None, kf, None)

        def load_pair(p):
            qtt = inp.tile([128, S], f32, tag="qtt")
            nc.sync.dma_start(qtt[:], qt[128 * p : 128 * (p + 1), :])
            ktt = inp.tile([128, S], f32, tag="ktt")
            nc.sync.dma_start(ktt[:], kt[128 * p : 128 * (p + 1), :])
            qtts[p], ktts[p] = qtt, ktt

        def emit_features(h, which):
            """Feature transform for head h ('q' or 'k' half), duplicated
            across both partition halves via PE column tiling."""
            p, e = h // 2, h % 2
            rb = 64 * e
            src = qtts[p] if which == "q" else ktts[p]
            dt_ = bf16 if cfg["score_bf16"] else f32
            hi = feat.tile([128, S], dt_, tag=which + "hi")
            lo = feat.tile([128, S], bf16, tag=which + "lo") if cfg["score_bf16"] else None
            pf = psum.tile([128, S], f32, tag="ps")
            for j in range(NCH):
                cs = slice(512 * j, 512 * (j + 1))
                for c in (0, 64):
                    nc.tensor.matmul(
                        pf[c : c + 64, cs],
                        lhsT=wtt[rb : rb + 64, :],
                        rhs=src[rb : rb + 64, cs],
                        start=True, stop=True,
                        tile_position=(rb, c),
                    )
            nc.scalar.activation(hi[:], pf[:], AF.Relu)
            if lo is not None:
                nc.vector.scalar_tensor_tensor(
                    lo[:], pf[:], 0.0, hi[:], OP.max, OP.subtract
                )
            st = feats.setdefault(h, {})
            st[which] = (hi, lo)

        def score_mms(ps_t, h, m, base, j):
            qhi, qlo = feats[h]["q"]
            khi, klo = feats[h]["k"]
            cs = slice(512 * j, 512 * (j + 1))
            mc = slice(128 * m, 128 * (m + 1))
            if cfg["score_bf16"]:
                terms = (
                    (qhi, khi, True, False),
                    (qhi, klo, False, False),
                    (qlo, khi, False, True),
                )
                for ql, kl, st, sp in terms:
                    nc.tensor.matmul(
                        ps_t[:, cs],
                        lhsT=ql[base : base + 64, mc],
                        rhs=kl[base : base + 64, cs],
                        start=st, stop=sp,
                        tile_position=(base, 0),
                    )
            else:
                nc.tensor.matmul(
                    ps_t[:, cs],
                    lhsT=qhi[base : base + 64, mc],
                    rhs=khi[base : base + 64, cs],
                    start=True, stop=True,
                    tile_position=(base, 0),
                )

        # ---- main schedule ----------------------------------------------
        load_pair(0)
        emit_features(0, "q")
        emit_features(0, "k")
        for h in range(HPC):
            # Points inside this head's score loop where the next head's
            # inputs/features are emitted (hides the feature->relu chain).
            nxt = h + 1
            prefetch = {}
            if nxt < HPC:
                if cfg["interleave_feat"]:
                    if nxt % 2 == 0:
                        prefetch[1] = ("load", nxt // 2)
                    prefetch[3] = ("feat", nxt, "q")
                    prefetch[5] = ("feat", nxt, "k")
                else:
                    prefetch[RT // 2 - 1] = ("all", nxt)

            if cfg["pack"]:
                for mp in range(RT // 2):
                    m0, m1 = 2 * mp, 2 * mp + 1
                    ps0 = psum.tile([128, S], f32, tag="ps")
                    ps1 = psum.tile([128, S], f32, tag="ps")
                    for j in range(NCH):
                        score_mms(ps0, h, m0, 0, j)
                        score_mms(ps1, h, m1, 64, j)
                    softmax_tail(ps0, h, m0)
                    softmax_tail(ps1, h, m1)
                    act = prefetch.get(mp)
                    if act:
                        if act[0] == "load":
                            load_pair(act[1])
                        elif act[0] == "feat":
                            emit_features(act[1], act[2])
                        else:
                            if nxt % 2 == 0:
                                load_pair(nxt // 2)
                            emit_features(nxt, "q")
                            emit_features(nxt, "k")
            else:
                for m in range(RT):
                    ps = psum.tile([128, S], f32, tag="ps")
                    for j in range(NCH):
                        score_mms(ps, h, m, 0, j)
                    softmax_tail(ps, h, m)
                    act = prefetch.get(m)
                    if act and act[0] == "all":
                        if nxt % 2 == 0:
                            load_pair(nxt // 2)
                        emit_features(nxt, "q")
                        emit_features(nxt, "k")

    nc.compile()
    return nc


def _cfg_key(cfg):
    return (
        cfg["pack"], cfg["copymax"], cfg["score_bf16"],
        cfg["mul_pattern"], cfg["interleave_feat"],
    )


def _get_program(cfg):
    key = _cfg_key(cfg)
    if key not in _CACHE:
        _CACHE[key] = _build_program(cfg)
    return _CACHE[key]


def make_in_maps(q, k, random_weights):
    """Host-side sharding/layout prep -> per-core input dicts."""
    q = np.asarray(q, dtype=np.float32)
    k = np.asarray(k, dtype=np.float32)
    w = np.asarray(random_weights, dtype=np.float32)
    # [B,H,S,D] -> [B*H, D, S]
    qT = np.ascontiguousarray(q.transpose(0, 1, 3, 2)).reshape(B * H, D, S)
    kT = np.ascontiguousarray(k.transpose(0, 1, 3, 2)).reshape(B * H, D, S)
    wt = np.ascontiguousarray(np.concatenate([w.T, w.T], axis=0))  # [128, D]
    in_maps = []
    for c in range(NCORES):
        qc = np.ascontiguousarray(qT[HPC * c : HPC * (c + 1)]).reshape(PAIRS * 128, S)
        kc = np.ascontiguousarray(kT[HPC * c : HPC * (c + 1)]).reshape(PAIRS * 128, S)
        in_maps.append({"qt": qc, "kt": kc, "wt": wt})
    return in_maps


def run_device(q, k, random_weights, cfg=None, trace=None):
    """Compile (cached), run on all 8 cores, return (attn_weights, results)."""
    from concourse.bass_utils import run_bass_kernel_spmd

    cfg = dict(CONFIG if cfg is None else cfg)
    if trace is not None:
        cfg["trace"] = trace
    nc = _get_program(cfg)
    in_maps = make_in_maps(q, k, random_weights)
    res = run_bass_kernel_spmd(
        nc, in_maps, core_ids=list(range(NCORES)), trace=cfg["trace"]
    )
    outs = [res.results[c]["out"].reshape(HPC, S, S) for c in range(NCORES)]
    attn_weights = np.concatenate(outs, axis=0).reshape(B, H, S, S)
    return attn_weights, res


def kernel(q, k, v, random_weights):
    attn_weights, _ = run_device(q, k, random_weights)
    attn_output = np.asarray(v, dtype=np.float32)
    return attn_output, attn_weights


# revision 5
# speedup vs baseline: 4.2675x; 1.0028x over previous
"""Trainium2 Bass kernel for nn_PerformerAttention.

reference math (B,H,S,D = 4,8,2048,64):
    qf = relu(q @ W.T); kf = relu(k @ W.T)          # [B,H,S,D]
    scores = qf @ kf.T                              # [B,H,S,S]
    attn_weights = softmax(scores, axis=-1)
    attn_output  = v * rowsum(attn_weights) == v    # softmax rows sum to 1
    returns (attn_output, attn_weights)

Sharding: B*H = 32 (b,h) pairs, 4 per core across 8 cores.  Host-side
layout prep only: q/k are transposed to [.., D, S] so the device never
needs on-chip transposes (matmul contracts over the partition dim).

Per-core device program (per head):
    feature transform qfT/kfT = relu(W^T.T @ qT/kT) on the PE, written
    to both SBUF partition halves (PE column tiling) so score matmuls
    can row-pack the half-empty K=64 PE array (fp32 matmul streams 4
    passes but drains once, so two row-groups overlap ~2x; bf16 is
    drain-bound and does not pack).  Feature work for head h+1 is
    emitted in the middle of head h's score loop to hide the serial
    feature->relu chain.
    for each 128-row tile of scores:
        s = qfT_chunk.T @ kfT            # PSUM [128, S] fp32, row-packed
        fused PSUM->SBUF copy + row-max (VectorE), exp+row-sum (ScalarE),
        multiply by 1/sum (engine per cfg), DMA out.
"""

import os
import numpy as np

B, H, S, D = 4, 8, 2048, 64
NCORES = 8
HPC = (B * H) // NCORES      # heads per core = 4
PAIRS = HPC // 2             # head pairs per core = 2
RT = S // 128                # 128-row score tiles per head = 16
NCH = S // 512               # 512-col matmul chunks per row tile = 4

# Tunables (test.py may override before calling kernel()).
CONFIG = {
    "pack": True,         # row-pack score matmuls across PE partition halves
    "copymax": True,      # fused PSUM->SBUF copy + row-max on VectorE
    "score_bf16": False,  # 3-term bf16-split score matmuls (else fp32 4-pass)
    "mul_pattern": "vvs", # normalize-mul engine per tile, cycled: g/v/s
    "interleave_feat": True,  # emit head h+1 features inside head h's scores
    "hiprio": True,       # schedule PSUM-releasing ops ahead of lagging muls
    "trace": False,       # request NTFF profile from the run
}

_CACHE = {}


def _build_program(cfg):
    from contextlib import ExitStack

    import concourse.bacc as bacc
    import concourse.mybir as mybir
    import concourse.tile as tile

    f32 = mybir.dt.float32
    bf16 = mybir.dt.bfloat16
    AF = mybir.ActivationFunctionType
    OP = mybir.AluOpType
    AX = mybir.AxisListType

    nc = bacc.Bacc(
        "TRN2",
        target_bir_lowering=False,
        debug=False,
        num_devices=NCORES,
    )

    qt = nc.dram_tensor("qt", [PAIRS * 128, S], f32, kind="ExternalInput").ap()
    kt = nc.dram_tensor("kt", [PAIRS * 128, S], f32, kind="ExternalInput").ap()
    wt = nc.dram_tensor("wt", [128, D], f32, kind="ExternalInput").ap()
    out = nc.dram_tensor("out", [HPC * S, S], f32, kind="ExternalOutput").ap()

    with tile.TileContext(nc) as tc, ExitStack() as ctx:
        const = ctx.enter_context(tc.tile_pool(name="const", bufs=1))
        inp = ctx.enter_context(tc.tile_pool(name="inp", bufs=2))
        feat = ctx.enter_context(tc.tile_pool(name="feat", bufs=2))
        psum = ctx.enter_context(tc.tile_pool(name="psum", bufs=2, space="PSUM"))
        work = ctx.enter_context(tc.tile_pool(name="work", bufs=4))
        stat = ctx.enter_context(tc.tile_pool(name="stat", bufs=8))
        outp = ctx.enter_context(tc.tile_pool(name="outp", bufs=4))

        wtt = const.tile([128, D], f32, tag="wtt")
        nc.sync.dma_start(wtt[:], wt[:, :])

        tile_idx = 0

        def softmax_tail(ps, h, m):
            """PSUM scores tile -> normalized SBUF tile -> DMA out."""
            nonlocal tile_idx
            negmax = stat.tile([128, 1], f32, tag="negmax")
            rowsum = stat.tile([128, 1], f32, tag="rowsum")
            rinv = stat.tile([128, 1], f32, tag="rinv")
            expt = work.tile([128, S], f32, tag="expt")
            if cfg["copymax"]:
                # sc = -scores (SBUF copy), negmax = min(-scores) = -rowmax.
                # Frees the PSUM tile after this single VectorE pass.  High
                # priority: the scheduler must prefer these over earlier
                # tiles' normalize-muls or the PSUM slots starve the PE.
                sc = work.tile([128, S], f32, tag="sc")
                from contextlib import nullcontext
                hp = tc.high_priority() if cfg["hiprio"] else nullcontext()
                with hp:
                    nc.vector.tensor_scalar(
                        sc[:], ps[:], -1.0, None, OP.mult, OP.min, accum_out=negmax[:]
                    )
                    # exp(-1*sc + negmax) = exp(s - max); fused row-sum.
                    nc.scalar.activation(
                        expt[:], sc[:], AF.Exp,
                        bias=negmax[:], scale=-1.0, accum_out=rowsum[:],
                    )
            else:
                nc.vector.reduce_max(negmax[:], ps[:], AX.X, negate=True)
                nc.scalar.activation(
                    expt[:], ps[:], AF.Exp,
                    bias=negmax[:], scale=1.0, accum_out=rowsum[:],
                )
            nc.vector.reciprocal(rinv[:], rowsum[:])
            ot = outp.tile([128, S], f32, tag="ot")
            eng = cfg["mul_pattern"][tile_idx % len(cfg["mul_pattern"])]
            if eng == "s":
                nc.scalar.activation(ot[:], expt[:], AF.Copy, bias=0.0, scale=rinv[:])
            elif eng == "g":
                nc.gpsimd.tensor_scalar(ot[:], expt[:], rinv[:], None, OP.mult, OP.bypass)
            else:
                nc.vector.tensor_scalar(ot[:], expt[:], rinv[:], None, OP.mult, OP.bypass)
            nc.sync.dma_start(out[h * S + 128 * m : h * S + 128 * (m + 1), :], ot[:])
            tile_idx += 1

        # ---- per-head feature state -------------------------------------
        qtts, ktts = {}, {}
        feats = {}  # h -> (qhi, qlo, khi, klo) or (qf, None, kf, None)

        def load_pair(p):
            qtt = inp.tile([128, S], f32, tag="qtt")
            nc.sync.dma_start(qtt[:], qt[128 * p : 128 * (p + 1), :])
            ktt = inp.tile([128, S], f32, tag="ktt")
            nc.sync.dma_start(ktt[:], kt[128 * p : 128 * (p + 1), :])
            qtts[p], ktts[p] = qtt, ktt

        def emit_features(h, which):
            """Feature transform for head h ('q' or 'k' half), duplicated
            across both partition halves via PE column tiling."""
            p, e = h // 2, h % 2
            rb = 64 * e
            src = qtts[p] if which == "q" else ktts[p]
            dt_ = bf16 if cfg["score_bf16"] else f32
            hi = feat.tile([128, S], dt_, tag=which + "hi")
            lo = feat.tile([128, S], bf16, tag=which + "lo") if cfg["score_bf16"] else None
            pf = psum.tile([128, S], f32, tag="ps")
            for j in range(NCH):
                cs = slice(512 * j, 512 * (j + 1))
                for c in (0, 64):
                    nc.tensor.matmul(
                        pf[c : c + 64, cs],
                        lhsT=wtt[rb : rb + 64, :],
                        rhs=src[rb : rb + 64, cs],
                        start=True, stop=True,
                        tile_position=(rb, c),
                    )
            nc.scalar.activation(hi[:], pf[:], AF.Relu)
            if lo is not None:
                nc.vector.scalar_tensor_tensor(
                    lo[:], pf[:], 0.0, hi[:], OP.max, OP.subtract
                )
            st = feats.setdefault(h, {})
            st[which] = (hi, lo)

        def score_mms(ps_t, h, m, base, j):
            qhi, qlo = feats[h]["q"]
            khi, klo = feats[h]["k"]
            cs = slice(512 * j, 512 * (j + 1))
            mc = slice(128 * m, 128 * (m + 1))
            if cfg["score_bf16"]:
                terms = (
                    (qhi, khi, True, False),
                    (qhi, klo, False, False),
                    (qlo, khi, False, True),
                )
                for ql, kl, st, sp in terms:
                    nc.tensor.matmul(
                        ps_t[:, cs],
                        lhsT=ql[base : base + 64, mc],
                        rhs=kl[base : base + 64, cs],
                        start=st, stop=sp,
                        tile_position=(base, 0),
                    )
            else:
                nc.tensor.matmul(
                    ps_t[:, cs],
                    lhsT=qhi[base : base + 64, mc],
                    rhs=khi[base : base + 64, cs],
                    start=True, stop=True,
                    tile_position=(base, 0),
                )

        # ---- main schedule ----------------------------------------------
        load_pair(0)
        emit_features(0, "q")
        emit_features(0, "k")
        for h in range(HPC):
            # Points inside this head's score loop where the next head's
            # inputs/features are emitted (hides the feature->relu chain).
            nxt = h + 1
            prefetch = {}
            if nxt < HPC:
                if cfg["interleave_feat"]:
                    if nxt % 2 == 0:
                        prefetch[1] = ("load", nxt // 2)
                    prefetch[3] = ("feat", nxt, "q")
                    prefetch[5] = ("feat", nxt, "k")
                else:
                    prefetch[RT // 2 - 1] = ("all", nxt)

            if cfg["pack"]:
                for mp in range(RT // 2):
                    m0, m1 = 2 * mp, 2 * mp + 1
                    ps0 = psum.tile([128, S], f32, tag="ps")
                    ps1 = psum.tile([128, S], f32, tag="ps")
                    for j in range(NCH):
                        score_mms(ps0, h, m0, 0, j)
                        score_mms(ps1, h, m1, 64, j)
                    softmax_tail(ps0, h, m0)
                    softmax_tail(ps1, h, m1)
                    act = prefetch.get(mp)
                    if act:
                        if act[0] == "load":
                            load_pair(act[1])
                        elif act[0] == "feat":
                            emit_features(act[1], act[2])
                        else:
                            if nxt % 2 == 0:
                                load_pair(nxt // 2)
                            emit_features(nxt, "q")
                            emit_features(nxt, "k")
            else:
                for m in range(RT):
                    ps = psum.tile([128, S], f32, tag="ps")
                    for j in range(NCH):
                        score_mms(ps, h, m, 0, j)
                    softmax_tail(ps, h, m)
                    act = prefetch.get(m)
                    if act and act[0] == "all":
                        if nxt % 2 == 0:
                            load_pair(nxt // 2)
                        emit_features(nxt, "q")
                        emit_features(nxt, "k")

    nc.compile()
    return nc


def _cfg_key(cfg):
    return (
        cfg["pack"], cfg["copymax"], cfg["score_bf16"],
        cfg["mul_pattern"], cfg["interleave_feat"], cfg["hiprio"],
    )


def _get_program(cfg):
    key = _cfg_key(cfg)
    if key not in _CACHE:
        _CACHE[key] = _build_program(cfg)
    return _CACHE[key]


def make_in_maps(q, k, random_weights):
    """Host-side sharding/layout prep -> per-core input dicts."""
    q = np.asarray(q, dtype=np.float32)
    k = np.asarray(k, dtype=np.float32)
    w = np.asarray(random_weights, dtype=np.float32)
    # [B,H,S,D] -> [B*H, D, S]
    qT = np.ascontiguousarray(q.transpose(0, 1, 3, 2)).reshape(B * H, D, S)
    kT = np.ascontiguousarray(k.transpose(0, 1, 3, 2)).reshape(B * H, D, S)
    wt = np.ascontiguousarray(np.concatenate([w.T, w.T], axis=0))  # [128, D]
    in_maps = []
    for c in range(NCORES):
        qc = np.ascontiguousarray(qT[HPC * c : HPC * (c + 1)]).reshape(PAIRS * 128, S)
        kc = np.ascontiguousarray(kT[HPC * c : HPC * (c + 1)]).reshape(PAIRS * 128, S)
        in_maps.append({"qt": qc, "kt": kc, "wt": wt})
    return in_maps


def run_device(q, k, random_weights, cfg=None, trace=None):
    """Compile (cached), run on all 8 cores, return (attn_weights, results)."""
    from concourse.bass_utils import run_bass_kernel_spmd

    cfg = dict(CONFIG if cfg is None else cfg)
    if trace is not None:
        cfg["trace"] = trace
    nc = _get_program(cfg)
    in_maps = make_in_maps(q, k, random_weights)
    res = run_bass_kernel_spmd(
        nc, in_maps, core_ids=list(range(NCORES)), trace=cfg["trace"]
    )
    outs = [res.results[c]["out"].reshape(HPC, S, S) for c in range(NCORES)]
    attn_weights = np.concatenate(outs, axis=0).reshape(B, H, S, S)
    return attn_weights, res


def kernel(q, k, v, random_weights):
    attn_weights, _ = run_device(q, k, random_weights)
    attn_output = np.asarray(v, dtype=np.float32)
    return attn_output, attn_weights
